# revision 21
# baseline (speedup 1.0000x reference)
"""Trainium2 Bass kernel for nn_Atten_Cross: cross-attention block.

Per-core (data-parallel over batch, B=8 == 8 cores):
  xf  = x[b]    [C=512, N=4096]
  cf  = cond[b] [C, N]
  Q   = wq @ xf          [64, N]   (o on partitions, duplicated to 128)
  K   = wk @ cf          [64, N]   (duplicated to 128)
  vT  = (gamma*wv @ cf).T [N, C]   (j on partitions, bf16)
  eT  = K.T-tile @ Q     [N(j), N(i)]  computed per (j-tile, i-block), f32r
  P   = exp(eT)          (no max subtraction; |e| <~ 10 so exp is safe)
  outT[i, c] = sum_j P[j,i] vT[j,c] / sum_j P[j,i]
  out_t = outT + xT (+ gamma*bv folded into xT host-side)
Host transposes out_t back to [C, H, W].
"""

import numpy as np

import concourse.bass as bass
import concourse.mybir as mybir
from concourse import bacc
from concourse.tile import TileContext
from concourse.bass_utils import run_bass_kernel_spmd

B, C, H, W = 8, 512, 64, 64
N = H * W          # 4096
CQK = C // 8       # 64
NB = 512           # i-block (columns per energy/PV pass)
NBLK = N // NB     # 8 i-blocks
NJT = N // 128     # 32 j-tiles
NPAIR = NJT // 2   # 16 row-packed pairs
NCT = C // 128     # 4 contraction tiles

F32 = mybir.dt.float32
F32R = mybir.dt.float32r
BF16 = mybir.dt.bfloat16
Ident = mybir.ActivationFunctionType.Identity
Exp = mybir.ActivationFunctionType.Exp
FP8 = mybir.dt.float8e4
DR = mybir.MatmulPerfMode.DoubleRow
EXP_BIAS = -7.0


def build():
    nc = bacc.Bacc("TRN2", target_bir_lowering=False)

    xf = nc.dram_tensor("xf", [C, N], BF16, kind="ExternalInput")
    cf = nc.dram_tensor("cf", [C, N], BF16, kind="ExternalInput")
    xtr = nc.dram_tensor("xtr", [N, C], F32, kind="ExternalInput")
    wqtd = nc.dram_tensor("wqtd", [C, 128], BF16, kind="ExternalInput")
    wktd = nc.dram_tensor("wktd", [C, 128], BF16, kind="ExternalInput")
    wvtg = nc.dram_tensor("wvtg", [C, C], BF16, kind="ExternalInput")
    bqd = nc.dram_tensor("bqd", [128, 1], F32, kind="ExternalInput")
    bkd = nc.dram_tensor("bkd", [128, 1], F32, kind="ExternalInput")
    out_t = nc.dram_tensor("out_t", [N, C], F32, kind="ExternalOutput")
    den_d = nc.dram_tensor("den_scratch", [NBLK, 4, 128], F32)

    with TileContext(nc) as tc:
        with (
            tc.tile_pool(name="wp", bufs=1) as wp,
            tc.tile_pool(name="qk", bufs=1) as qk,
            tc.tile_pool(name="vp", bufs=1) as vp,
            tc.tile_pool(name="io", bufs=1) as io,
        ):
            # ---- weights / constants ----
            wq_t = wp.tile([128, NCT, 128], BF16)
            wk_t = wp.tile([128, NCT, 128], BF16)
            wvg_t = wp.tile([128, NCT, C], BF16)
            for c in range(NCT):
                nc.sync.dma_start(out=wq_t[:, c, :], in_=wqtd[c * 128:(c + 1) * 128, :])
                nc.sync.dma_start(out=wk_t[:, c, :], in_=wktd[c * 128:(c + 1) * 128, :])
                nc.sync.dma_start(out=wvg_t[:, c, :], in_=wvtg[c * 128:(c + 1) * 128, :])
            bq_t = wp.tile([128, 1], F32)
            bk_t = wp.tile([128, 1], F32)
            nc.sync.dma_start(out=bq_t, in_=bqd[:])
            nc.sync.dma_start(out=bk_t, in_=bkd[:])
            ones_0 = wp.tile([128, 1], F32)
            nc.vector.memset(ones_0, 1.0)
            ones_f = wp.tile([128, 1], F32R)
            nc.vector.tensor_copy(ones_f[:], ones_0[:])
            ebias_t = wp.tile([128, 1], F32)
            nc.vector.memset(ebias_t, EXP_BIAS)

            q2 = qk.tile([128, N], BF16)   # [o-dup, i]
            k2 = qk.tile([128, N], BF16)   # [o-dup, j]
            v8 = [vp.tile([128, 2, NB], FP8, tag=f"v{t}", name=f"v{t}") for t in range(NPAIR)]

            # ================= projections =================
            with (
                tc.tile_pool(name="cfp", bufs=2) as cfp,
                tc.tile_pool(name="xfp", bufs=2) as xfp,
                tc.tile_pool(name="pjq", bufs=1, space="PSUM") as pjq,
                tc.tile_pool(name="pjv", bufs=2, space="PSUM") as pjv,
            ):
                for h in range(2):
                    csl = slice(h * 2048, (h + 1) * 2048)
                    cf_t = []
                    for c in range(NCT):
                        t = cfp.tile([128, 2048], BF16, tag=f"c{c}", name=f"cf{c}_{h}")
                        for u in range(4):
                            nc.sync.dma_start(
                                out=t[:, u * NB:(u + 1) * NB],
                                in_=cf[c * 128:(c + 1) * 128,
                                       h * 2048 + u * NB: h * 2048 + (u + 1) * NB],
                            )
                        cf_t.append(t)
                    # K projection: 4 column-blocks of this half
                    for ibl in range(4):
                        blk = h * 4 + ibl
                        psk = pjq.tile([128, NB], F32, tag=f"q{ibl}", name=f"psk{blk}")
                        for c in range(NCT):
                            nc.tensor.matmul(
                                psk[:],
                                wk_t[:, c, :],
                                cf_t[c][:, ibl * NB:(ibl + 1) * NB],
                                start=(c == 0), stop=(c == NCT - 1),
                            )
                        nc.scalar.activation(
                            out=k2[:, blk * NB:(blk + 1) * NB], in_=psk[:],
                            func=Ident, bias=bk_t[:], scale=1.0,
                        )
                    # vT projection: 16 j-tiles of this half
                    for jl in range(16):
                        j = h * 16 + jl
                        psv = pjv.tile([128, NB], F32, tag="pv", name=f"psv{j}")
                        for c in range(NCT):
                            nc.tensor.matmul(
                                psv[:],
                                cf_t[c][:, jl * 128:(jl + 1) * 128],
                                wvg_t[:, c, :],
                                start=(c == 0), stop=(c == NCT - 1),
                            )
                        nc.scalar.copy(v8[j // 2][:, j % 2, :], psv[:])
                    # Q projection: stream xf, accumulate 4 blocks in PSUM
                    psq = [
                        pjq.tile([128, NB], F32, tag=f"q{ibl}", name=f"psq{h}_{ibl}")
                        for ibl in range(4)
                    ]
                    for c in range(NCT):
                        xs = xfp.tile([128, 2048], BF16, tag="xs", name=f"xs{h}_{c}")
                        nc.sync.dma_start(out=xs, in_=xf[c * 128:(c + 1) * 128, csl])
                        for ibl in range(4):
                            nc.tensor.matmul(
                                psq[ibl][:],
                                wq_t[:, c, :],
                                xs[:, ibl * NB:(ibl + 1) * NB],
                                start=(c == 0), stop=(c == NCT - 1),
                            )
                    for ibl in range(4):
                        blk = h * 4 + ibl
                        nc.scalar.activation(
                            out=q2[:, blk * NB:(blk + 1) * NB], in_=psq[ibl][:],
                            func=Ident, bias=bq_t[:], scale=1.0,
                        )

            # ================= attention =================
            with (
                tc.tile_pool(name="pe", bufs=2, space="PSUM") as pe_pool,
                tc.tile_pool(name="po", bufs=3, space="PSUM") as po_pool,
                tc.tile_pool(name="pd", bufs=1, space="PSUM") as pd_pool,
                tc.tile_pool(name="pp", bufs=2) as pp,
                tc.tile_pool(name="att_io", bufs=4) as aio,
            ):
                p_cur = [None] * NPAIR  # p-tiles for the block being PV'd
                acc_cur = [None, None]

                def emit_energy_pair(b, t):
                    """energy + exp + denominator partial for pair (2t, 2t+1)."""
                    jA, jB = 2 * t, 2 * t + 1
                    isl = slice(b * NB, (b + 1) * NB)
                    pe = pe_pool.tile([128, 2 * NB], F32, tag="e", name=f"pe{b}_{t}")
                    nc.tensor.matmul(
                        pe[:, 0:NB],
                        k2[0:64, jA * 128:(jA + 1) * 128],
                        q2[0:64, isl],
                    )
                    nc.tensor.matmul(
                        pe[:, NB:2 * NB],
                        k2[64:128, jB * 128:(jB + 1) * 128],
                        q2[64:128, isl],
                    )
                    pt = pp.tile([128, 2 * NB], FP8, tag=f"p{t}", name=f"p{b}_{t}")
                    nc.scalar.activation(out=pt[:], in_=pe[:], func=Exp, bias=ebias_t[:], scale=1.0)
                    p_cur[t] = pt
                    # denominator accumulation split between DVE and GPSIMD
                    if t == 0:
                        acc = aio.tile([128, NB], F32R, tag="acc", name=f"acc{b}")
                        acc_cur[0] = acc
                        nc.vector.tensor_add(acc[:], pt[:, 0:NB], pt[:, NB:2 * NB])
                    elif t == 1:
                        accb = aio.tile([128, NB], F32R, tag="accb", name=f"accb{b}")
                        acc_cur[1] = accb
                        nc.gpsimd.tensor_add(accb[:], pt[:, 0:NB], pt[:, NB:2 * NB])
                    elif t < 7:
                        accb = acc_cur[1]
                        nc.gpsimd.tensor_add(accb[:], accb[:], pt[:, 0:NB])
                        nc.gpsimd.tensor_add(accb[:], accb[:], pt[:, NB:2 * NB])
                    else:
                        acc = acc_cur[0]
                        nc.vector.tensor_add(acc[:], acc[:], pt[:, 0:NB])
                        nc.vector.tensor_add(acc[:], acc[:], pt[:, NB:2 * NB])

                def emit_denominator(b, acc, accb):
                    """Fold accs over partitions (ones-matmuls), scatter, reciprocal."""
                    pdn = pd_pool.tile([1, NB], F32, tag="d", name=f"pd{b}")
                    nc.tensor.matmul(pdn[:], ones_f[:], acc[:], start=True, stop=False)
                    nc.tensor.matmul(pdn[:], ones_f[:], accb[:], start=False, stop=True)
                    den_sb = aio.tile([1, NB], F32, tag="den", name=f"den{b}")
                    nc.scalar.copy(den_sb[:], pdn[:])
                    # partition-scatter via DRAM bounce: den[f*128+p] -> rcp[p, f]
                    nc.sync.dma_start(
                        out=den_d[b:b + 1].rearrange("o f p -> o (f p)"),
                        in_=den_sb[:],
                    )
                    rcp = aio.tile([128, 4], F32, tag="rcp", name=f"rcp{b}")
                    nc.sync.dma_start(
                        out=rcp[:], in_=den_d[b].rearrange("f p -> p f"),
                    )
                    rc4 = aio.tile([128, 4], F32, tag="rc4", name=f"rc4{b}")
                    nc.vector.reciprocal(rc4[:], rcp[:])
                    return rc4

                # prologue: block 0 energy
                for t in range(NPAIR):
                    emit_energy_pair(0, t)
                p_live = list(p_cur)
                acc_live, accb_live = acc_cur[0], acc_cur[1]

                for b in range(NBLK):
                    rc4 = None
                    # prefetch residual tiles for this block
                    xt_tiles = []
                    for it in range(4):
                        row = (b * 4 + it) * 128
                        xt = aio.tile([128, C], F32, tag="xt", name=f"xt{b}_{it}")
                        nc.sync.dma_start(out=xt, in_=xtr[row:row + 128, :])
                        xt_tiles.append(xt)

                    nxt = iter(range(NPAIR)) if b + 1 < NBLK else iter(())
                    pos = 0
                    for it in range(4):
                        po = po_pool.tile([128, NB], F32, tag="o", name=f"po{b}_{it}")
                        for k in range(NPAIR):
                            lhs = p_live[k].rearrange("p (h i) -> p h i", h=2)[
                                :, :, it * 128:(it + 1) * 128]
                            nc.tensor.matmul(
                                po[:], lhs, v8[k][:],
                                start=(k == 0), stop=(k == NPAIR - 1),
                                perf_mode=DR,
                            )
                            pos += 1
                            if pos % 4 == 0:
                                t_nxt = next(nxt, None)
                                if t_nxt is not None:
                                    emit_energy_pair(b + 1, t_nxt)
                        if rc4 is None:
                            rc4 = emit_denominator(b, acc_live, accb_live)
                        ot = aio.tile([128, C], F32, tag="ot", name=f"ot{b}_{it}")
                        nc.vector.scalar_tensor_tensor(
                            out=ot[:], in0=po[:], scalar=rc4[:, it:it + 1], in1=xt_tiles[it][:],
                            op0=mybir.AluOpType.mult, op1=mybir.AluOpType.add,
                        )
                        row = (b * 4 + it) * 128
                        nc.sync.dma_start(out=out_t[row:row + 128, :], in_=ot[:])
                    p_live = list(p_cur)
                    acc_live, accb_live = acc_cur[0], acc_cur[1]

    nc.finalize()
    return nc


_CACHE = {}


def _get_nc():
    if "nc" not in _CACHE:
        _CACHE["nc"] = build()
    return _CACHE["nc"]


def run(x, cond, wq, bq, wk, bk, wv, bv, gamma, trace=False, tmpdir=None):
    x = np.asarray(x, np.float32)
    cond = np.asarray(cond, np.float32)
    wq = np.asarray(wq, np.float32)
    bq = np.asarray(bq, np.float32)
    wk = np.asarray(wk, np.float32)
    bk = np.asarray(bk, np.float32)
    wv = np.asarray(wv, np.float32)
    bv = np.asarray(bv, np.float32)
    g = float(np.asarray(gamma, np.float32).reshape(-1)[0])

    import ml_dtypes
    BF = ml_dtypes.bfloat16

    wqtd = np.ascontiguousarray(np.concatenate([wq.T, wq.T], axis=1)).astype(BF)
    wktd = np.ascontiguousarray(np.concatenate([wk.T, wk.T], axis=1)).astype(BF)
    wvtg = np.ascontiguousarray((g * wv).T).astype(BF)                  # [C, C]
    bqd = np.ascontiguousarray(np.tile(bq, 2)[:, None])                 # [128, 1]
    bkd = np.ascontiguousarray(np.tile(bk, 2)[:, None])
    gbv = (g * bv)[None, :]                                             # [1, C]

    in_maps = []
    for b in range(B):
        xf32 = np.ascontiguousarray(x[b].reshape(C, N))
        xfb = xf32.astype(BF)
        cfb = np.ascontiguousarray(cond[b].reshape(C, N)).astype(BF)
        xtrb = np.ascontiguousarray(xf32.T + gbv)
        in_maps.append({
            "xf": xfb, "cf": cfb, "xtr": xtrb,
            "wqtd": wqtd, "wktd": wktd, "wvtg": wvtg,
            "bqd": bqd, "bkd": bkd,
        })

    nc = _get_nc()
    res = run_bass_kernel_spmd(
        nc, in_maps, list(range(B)), trace=trace, tmpdir=tmpdir,
    )
    out = np.empty((B, C, H, W), np.float32)
    for b in range(B):
        out[b] = res.results[b]["out_t"].T.reshape(C, H, W)
    return out, res


def kernel(**inputs):
    out, _ = run(**inputs)
    return out


# revision 22
# speedup vs baseline: 1.0069x; 1.0069x over previous
"""Trainium2 Bass kernel for nn_Atten_Cross: cross-attention block.

Per-core (data-parallel over batch, B=8 == 8 cores):
  xf  = x[b]    [C=512, N=4096]
  cf  = cond[b] [C, N]
  Q   = wq @ xf          [64, N]   (o on partitions, duplicated to 128)
  K   = wk @ cf          [64, N]   (duplicated to 128)
  vT  = (gamma*wv @ cf).T [N, C]   (j on partitions, bf16)
  eT  = K.T-tile @ Q     [N(j), N(i)]  computed per (j-tile, i-block), f32r
  P   = exp(eT)          (no max subtraction; |e| <~ 10 so exp is safe)
  outT[i, c] = sum_j P[j,i] vT[j,c] / sum_j P[j,i]
  out_t = outT + xT (+ gamma*bv folded into xT host-side)
Host transposes out_t back to [C, H, W].
"""

import numpy as np

import concourse.bass as bass
import concourse.mybir as mybir
from concourse import bacc
from concourse.tile import TileContext
from concourse.bass_utils import run_bass_kernel_spmd

B, C, H, W = 8, 512, 64, 64
N = H * W          # 4096
CQK = C // 8       # 64
NB = 512           # i-block (columns per energy/PV pass)
NBLK = N // NB     # 8 i-blocks
NJT = N // 128     # 32 j-tiles
NPAIR = NJT // 2   # 16 row-packed pairs
NCT = C // 128     # 4 contraction tiles

F32 = mybir.dt.float32
F32R = mybir.dt.float32r
BF16 = mybir.dt.bfloat16
Ident = mybir.ActivationFunctionType.Identity
Exp = mybir.ActivationFunctionType.Exp
FP8 = mybir.dt.float8e4
DR = mybir.MatmulPerfMode.DoubleRow
EXP_BIAS = -7.0


def build():
    nc = bacc.Bacc("TRN2", target_bir_lowering=False)

    xf = nc.dram_tensor("xf", [C, N], BF16, kind="ExternalInput")
    cf = nc.dram_tensor("cf", [C, N], BF16, kind="ExternalInput")
    xtr = nc.dram_tensor("xtr", [N, C], F32, kind="ExternalInput")
    wqtd = nc.dram_tensor("wqtd", [C, 128], BF16, kind="ExternalInput")
    wktd = nc.dram_tensor("wktd", [C, 128], BF16, kind="ExternalInput")
    wvtg = nc.dram_tensor("wvtg", [C, C], BF16, kind="ExternalInput")
    bqd = nc.dram_tensor("bqd", [128, 1], F32, kind="ExternalInput")
    bkd = nc.dram_tensor("bkd", [128, 1], F32, kind="ExternalInput")
    out_t = nc.dram_tensor("out_t", [N, C], F32, kind="ExternalOutput")
    den_d = nc.dram_tensor("den_scratch", [NBLK, 4, 128], F32)

    with TileContext(nc) as tc:
        with (
            tc.tile_pool(name="wp", bufs=1) as wp,
            tc.tile_pool(name="qk", bufs=1) as qk,
            tc.tile_pool(name="vp", bufs=1) as vp,
            tc.tile_pool(name="io", bufs=1) as io,
        ):
            # ---- weights / constants ----
            wq_t = wp.tile([128, NCT, 128], BF16)
            wk_t = wp.tile([128, NCT, 128], BF16)
            wvg_t = wp.tile([128, NCT, C], BF16)
            for c in range(NCT):
                nc.sync.dma_start(out=wq_t[:, c, :], in_=wqtd[c * 128:(c + 1) * 128, :])
                nc.sync.dma_start(out=wk_t[:, c, :], in_=wktd[c * 128:(c + 1) * 128, :])
                nc.sync.dma_start(out=wvg_t[:, c, :], in_=wvtg[c * 128:(c + 1) * 128, :])
            bq_t = wp.tile([128, 1], F32)
            bk_t = wp.tile([128, 1], F32)
            nc.sync.dma_start(out=bq_t, in_=bqd[:])
            nc.sync.dma_start(out=bk_t, in_=bkd[:])
            ones_0 = wp.tile([128, 1], F32)
            nc.vector.memset(ones_0, 1.0)
            ones_f = wp.tile([128, 1], F32R)
            nc.vector.tensor_copy(ones_f[:], ones_0[:])
            ebias_t = wp.tile([128, 1], F32)
            nc.vector.memset(ebias_t, EXP_BIAS)

            q2 = qk.tile([128, N], BF16)   # [o-dup, i]
            k2 = qk.tile([128, N], BF16)   # [o-dup, j]
            v8 = [vp.tile([128, 2, NB], FP8, tag=f"v{t}", name=f"v{t}") for t in range(NPAIR)]

            # ================= projections =================
            with (
                tc.tile_pool(name="cfp", bufs=2) as cfp,
                tc.tile_pool(name="xfp", bufs=2) as xfp,
                tc.tile_pool(name="pjq", bufs=1, space="PSUM") as pjq,
                tc.tile_pool(name="pjv", bufs=2, space="PSUM") as pjv,
            ):
                for h in range(2):
                    csl = slice(h * 2048, (h + 1) * 2048)
                    cf_t = []
                    for c in range(NCT):
                        t = cfp.tile([128, 2048], BF16, tag=f"c{c}", name=f"cf{c}_{h}")
                        for u in range(4):
                            nc.sync.dma_start(
                                out=t[:, u * NB:(u + 1) * NB],
                                in_=cf[c * 128:(c + 1) * 128,
                                       h * 2048 + u * NB: h * 2048 + (u + 1) * NB],
                            )
                        cf_t.append(t)
                    # K projection: 4 column-blocks of this half
                    for ibl in range(4):
                        blk = h * 4 + ibl
                        psk = pjq.tile([128, NB], F32, tag=f"q{ibl}", name=f"psk{blk}")
                        for c in range(NCT):
                            nc.tensor.matmul(
                                psk[:],
                                wk_t[:, c, :],
                                cf_t[c][:, ibl * NB:(ibl + 1) * NB],
                                start=(c == 0), stop=(c == NCT - 1),
                            )
                        nc.scalar.activation(
                            out=k2[:, blk * NB:(blk + 1) * NB], in_=psk[:],
                            func=Ident, bias=bk_t[:], scale=1.0,
                        )
                    # vT projection: 16 j-tiles of this half
                    for jl in range(16):
                        j = h * 16 + jl
                        psv = pjv.tile([128, NB], F32, tag="pv", name=f"psv{j}")
                        for c in range(NCT):
                            nc.tensor.matmul(
                                psv[:],
                                cf_t[c][:, jl * 128:(jl + 1) * 128],
                                wvg_t[:, c, :],
                                start=(c == 0), stop=(c == NCT - 1),
                            )
                        nc.scalar.copy(v8[j // 2][:, j % 2, :], psv[:])
                    # Q projection: stream xf, accumulate 4 blocks in PSUM
                    psq = [
                        pjq.tile([128, NB], F32, tag=f"q{ibl}", name=f"psq{h}_{ibl}")
                        for ibl in range(4)
                    ]
                    for c in range(NCT):
                        xs = xfp.tile([128, 2048], BF16, tag="xs", name=f"xs{h}_{c}")
                        nc.sync.dma_start(out=xs, in_=xf[c * 128:(c + 1) * 128, csl])
                        for ibl in range(4):
                            nc.tensor.matmul(
                                psq[ibl][:],
                                wq_t[:, c, :],
                                xs[:, ibl * NB:(ibl + 1) * NB],
                                start=(c == 0), stop=(c == NCT - 1),
                            )
                    for ibl in range(4):
                        blk = h * 4 + ibl
                        nc.scalar.activation(
                            out=q2[:, blk * NB:(blk + 1) * NB], in_=psq[ibl][:],
                            func=Ident, bias=bq_t[:], scale=1.0,
                        )

            # ================= attention =================
            with (
                tc.tile_pool(name="pe", bufs=2, space="PSUM") as pe_pool,
                tc.tile_pool(name="po", bufs=3, space="PSUM") as po_pool,
                tc.tile_pool(name="pd", bufs=1, space="PSUM") as pd_pool,
                tc.tile_pool(name="pp", bufs=2) as pp,
                tc.tile_pool(name="att_io", bufs=4) as aio,
            ):
                p_cur = [None] * NPAIR  # p-tiles for the block being PV'd
                acc_cur = [None]

                def emit_energy_pair(b, t):
                    """energy + exp + denominator partial for pair (2t, 2t+1)."""
                    jA, jB = 2 * t, 2 * t + 1
                    isl = slice(b * NB, (b + 1) * NB)
                    pe = pe_pool.tile([128, 2 * NB], F32, tag="e", name=f"pe{b}_{t}")
                    nc.tensor.matmul(
                        pe[:, 0:NB],
                        k2[0:64, jA * 128:(jA + 1) * 128],
                        q2[0:64, isl],
                    )
                    nc.tensor.matmul(
                        pe[:, NB:2 * NB],
                        k2[64:128, jB * 128:(jB + 1) * 128],
                        q2[64:128, isl],
                    )
                    pt = pp.tile([128, 2 * NB], FP8, tag=f"p{t}", name=f"p{b}_{t}")
                    nc.scalar.activation(out=pt[:], in_=pe[:], func=Exp, bias=ebias_t[:], scale=1.0)
                    p_cur[t] = pt
                    # denominator accumulation split between DVE and GPSIMD
                    if t == 0:
                        acc = aio.tile([128, NB], F32R, tag="acc", name=f"acc{b}")
                        acc_cur[0] = acc
                        nc.vector.tensor_add(acc[:], pt[:, 0:NB], pt[:, NB:2 * NB])
                    else:
                        acc = acc_cur[0]
                        nc.vector.tensor_add(acc[:], acc[:], pt[:, 0:NB])
                        nc.vector.tensor_add(acc[:], acc[:], pt[:, NB:2 * NB])

                def emit_denominator(b, acc):
                    """Fold acc over partitions (ones-matmul), scatter, reciprocal."""
                    pdn = pd_pool.tile([1, NB], F32, tag="d", name=f"pd{b}")
                    nc.tensor.matmul(pdn[:], ones_f[:], acc[:])
                    den_sb = aio.tile([1, NB], F32, tag="den", name=f"den{b}")
                    nc.vector.tensor_copy(den_sb[:], pdn[:])
                    # partition-scatter via DRAM bounce: den[f*128+p] -> rcp[p, f]
                    nc.sync.dma_start(
                        out=den_d[b:b + 1].rearrange("o f p -> o (f p)"),
                        in_=den_sb[:],
                    )
                    rcp = aio.tile([128, 4], F32, tag="rcp", name=f"rcp{b}")
                    nc.sync.dma_start(
                        out=rcp[:], in_=den_d[b].rearrange("f p -> p f"),
                    )
                    rc4 = aio.tile([128, 4], F32, tag="rc4", name=f"rc4{b}")
                    nc.vector.reciprocal(rc4[:], rcp[:])
                    return rc4

                # prologue: block 0 energy
                for t in range(NPAIR):
                    emit_energy_pair(0, t)
                p_live = list(p_cur)
                acc_live = acc_cur[0]

                for b in range(NBLK):
                    rc4 = None
                    # prefetch residual tiles for this block
                    xt_tiles = []
                    for it in range(4):
                        row = (b * 4 + it) * 128
                        xt = aio.tile([128, C], F32, tag="xt", name=f"xt{b}_{it}")
                        nc.sync.dma_start(out=xt, in_=xtr[row:row + 128, :])
                        xt_tiles.append(xt)

                    nxt = iter(range(NPAIR)) if b + 1 < NBLK else iter(())
                    pos = 0
                    for it in range(4):
                        po = po_pool.tile([128, NB], F32, tag="o", name=f"po{b}_{it}")
                        for k in range(NPAIR):
                            lhs = p_live[k].rearrange("p (h i) -> p h i", h=2)[
                                :, :, it * 128:(it + 1) * 128]
                            nc.tensor.matmul(
                                po[:], lhs, v8[k][:],
                                start=(k == 0), stop=(k == NPAIR - 1),
                                perf_mode=DR,
                            )
                            pos += 1
                            if pos % 4 == 0:
                                t_nxt = next(nxt, None)
                                if t_nxt is not None:
                                    emit_energy_pair(b + 1, t_nxt)
                        if rc4 is None:
                            rc4 = emit_denominator(b, acc_live)
                        ot = aio.tile([128, C], F32, tag="ot", name=f"ot{b}_{it}")
                        nc.vector.scalar_tensor_tensor(
                            out=ot[:], in0=po[:], scalar=rc4[:, it:it + 1], in1=xt_tiles[it][:],
                            op0=mybir.AluOpType.mult, op1=mybir.AluOpType.add,
                        )
                        row = (b * 4 + it) * 128
                        nc.sync.dma_start(out=out_t[row:row + 128, :], in_=ot[:])
                    p_live = list(p_cur)
                    acc_live = acc_cur[0]

    nc.finalize()
    return nc


_CACHE = {}


def _get_nc():
    if "nc" not in _CACHE:
        _CACHE["nc"] = build()
    return _CACHE["nc"]


def run(x, cond, wq, bq, wk, bk, wv, bv, gamma, trace=False, tmpdir=None):
    x = np.asarray(x, np.float32)
    cond = np.asarray(cond, np.float32)
    wq = np.asarray(wq, np.float32)
    bq = np.asarray(bq, np.float32)
    wk = np.asarray(wk, np.float32)
    bk = np.asarray(bk, np.float32)
    wv = np.asarray(wv, np.float32)
    bv = np.asarray(bv, np.float32)
    g = float(np.asarray(gamma, np.float32).reshape(-1)[0])

    import ml_dtypes
    BF = ml_dtypes.bfloat16

    wqtd = np.ascontiguousarray(np.concatenate([wq.T, wq.T], axis=1)).astype(BF)
    wktd = np.ascontiguousarray(np.concatenate([wk.T, wk.T], axis=1)).astype(BF)
    wvtg = np.ascontiguousarray((g * wv).T).astype(BF)                  # [C, C]
    bqd = np.ascontiguousarray(np.tile(bq, 2)[:, None])                 # [128, 1]
    bkd = np.ascontiguousarray(np.tile(bk, 2)[:, None])
    gbv = (g * bv)[None, :]                                             # [1, C]

    in_maps = []
    for b in range(B):
        xf32 = np.ascontiguousarray(x[b].reshape(C, N))
        xfb = xf32.astype(BF)
        cfb = np.ascontiguousarray(cond[b].reshape(C, N)).astype(BF)
        xtrb = np.ascontiguousarray(xf32.T + gbv)
        in_maps.append({
            "xf": xfb, "cf": cfb, "xtr": xtrb,
            "wqtd": wqtd, "wktd": wktd, "wvtg": wvtg,
            "bqd": bqd, "bkd": bkd,
        })

    nc = _get_nc()
    res = run_bass_kernel_spmd(
        nc, in_maps, list(range(B)), trace=trace, tmpdir=tmpdir,
    )
    out = np.empty((B, C, H, W), np.float32)
    for b in range(B):
        out[b] = res.results[b]["out_t"].T.reshape(C, H, W)
    return out, res


def kernel(**inputs):
    out, _ = run(**inputs)
    return out


# revision 23
# speedup vs baseline: 1.0882x; 1.0808x over previous
"""Trainium2 Bass kernel for nn_Atten_Cross: cross-attention block.

Per-core (data-parallel over batch, B=8 == 8 cores):
  xf  = x[b]    [C=512, N=4096]
  cf  = cond[b] [C, N]
  Q   = wq @ xf          [64, N]   (o on partitions, duplicated to 128)
  K   = wk @ cf          [64, N]   (duplicated to 128)
  vT  = (gamma*wv @ cf).T [N, C]   (j on partitions, bf16)
  eT  = K.T-tile @ Q     [N(j), N(i)]  computed per (j-tile, i-block), f32r
  P   = exp(eT)          (no max subtraction; |e| <~ 10 so exp is safe)
  outT[i, c] = sum_j P[j,i] vT[j,c] / sum_j P[j,i]
  out_t = outT + xT (+ gamma*bv folded into xT host-side)
Host transposes out_t back to [C, H, W].
"""

import numpy as np

import concourse.bass as bass
import concourse.mybir as mybir
from concourse import bacc
from concourse.tile import TileContext
from concourse.bass_utils import run_bass_kernel_spmd

B, C, H, W = 8, 512, 64, 64
N = H * W          # 4096
CQK = C // 8       # 64
NB = 512           # i-block (columns per energy/PV pass)
NBLK = N // NB     # 8 i-blocks
NJT = N // 128     # 32 j-tiles
NPAIR = NJT // 2   # 16 row-packed pairs
NCT = C // 128     # 4 contraction tiles

F32 = mybir.dt.float32
F32R = mybir.dt.float32r
BF16 = mybir.dt.bfloat16
Ident = mybir.ActivationFunctionType.Identity
Exp = mybir.ActivationFunctionType.Exp
FP8 = mybir.dt.float8e4
DR = mybir.MatmulPerfMode.DoubleRow
EXP_BIAS = -7.0


def build():
    nc = bacc.Bacc("TRN2", target_bir_lowering=False)

    xf = nc.dram_tensor("xf", [C, N], BF16, kind="ExternalInput")
    cf = nc.dram_tensor("cf", [C, N], BF16, kind="ExternalInput")
    xtr = nc.dram_tensor("xtr", [N, C], F32, kind="ExternalInput")
    wqtd = nc.dram_tensor("wqtd", [C, 128], BF16, kind="ExternalInput")
    wktd = nc.dram_tensor("wktd", [C, 128], BF16, kind="ExternalInput")
    wvtg = nc.dram_tensor("wvtg", [C, C], BF16, kind="ExternalInput")
    bqd = nc.dram_tensor("bqd", [128, 1], F32, kind="ExternalInput")
    bkd = nc.dram_tensor("bkd", [128, 1], F32, kind="ExternalInput")
    out_t = nc.dram_tensor("out_t", [N, C], F32, kind="ExternalOutput")
    den_d = nc.dram_tensor("den_scratch", [NBLK, 4, 128], F32)

    with TileContext(nc) as tc:
        with (
            tc.tile_pool(name="wp", bufs=1) as wp,
            tc.tile_pool(name="qk", bufs=1) as qk,
            tc.tile_pool(name="vp", bufs=1) as vp,
            tc.tile_pool(name="io", bufs=1) as io,
        ):
            # ---- weights / constants ----
            wq_t = wp.tile([128, NCT, 128], BF16)
            wk_t = wp.tile([128, NCT, 128], BF16)
            wvg_t = wp.tile([128, NCT, C], BF16)
            for c in range(NCT):
                nc.sync.dma_start(out=wq_t[:, c, :], in_=wqtd[c * 128:(c + 1) * 128, :])
                nc.sync.dma_start(out=wk_t[:, c, :], in_=wktd[c * 128:(c + 1) * 128, :])
                nc.sync.dma_start(out=wvg_t[:, c, :], in_=wvtg[c * 128:(c + 1) * 128, :])
            bq_t = wp.tile([128, 1], F32)
            bk_t = wp.tile([128, 1], F32)
            nc.sync.dma_start(out=bq_t, in_=bqd[:])
            nc.sync.dma_start(out=bk_t, in_=bkd[:])
            ones_0 = wp.tile([128, 1], F32)
            nc.vector.memset(ones_0, 1.0)
            ones_f = wp.tile([128, 1], F32R)
            nc.vector.tensor_copy(ones_f[:], ones_0[:])
            ones_8 = wp.tile([128, 1], FP8)
            nc.vector.tensor_copy(ones_8[:], ones_0[:])
            ebias_t = wp.tile([128, 1], F32)
            nc.vector.memset(ebias_t, EXP_BIAS)

            q2 = qk.tile([128, N], BF16)   # [o-dup, i]
            k2 = qk.tile([128, N], BF16)   # [o-dup, j]
            v8 = [vp.tile([128, 2, NB], FP8, tag=f"v{t}", name=f"v{t}") for t in range(NPAIR)]

            # ================= projections =================
            with (
                tc.tile_pool(name="cfp", bufs=2) as cfp,
                tc.tile_pool(name="xfp", bufs=2) as xfp,
                tc.tile_pool(name="pjq", bufs=1, space="PSUM") as pjq,
                tc.tile_pool(name="pjv", bufs=2, space="PSUM") as pjv,
            ):
                for h in range(2):
                    csl = slice(h * 2048, (h + 1) * 2048)
                    cf_t = []
                    for c in range(NCT):
                        t = cfp.tile([128, 2048], BF16, tag=f"c{c}", name=f"cf{c}_{h}")
                        for u in range(4):
                            nc.sync.dma_start(
                                out=t[:, u * NB:(u + 1) * NB],
                                in_=cf[c * 128:(c + 1) * 128,
                                       h * 2048 + u * NB: h * 2048 + (u + 1) * NB],
                            )
                        cf_t.append(t)
                    # K projection: 4 column-blocks of this half
                    for ibl in range(4):
                        blk = h * 4 + ibl
                        psk = pjq.tile([128, NB], F32, tag=f"q{ibl}", name=f"psk{blk}")
                        for c in range(NCT):
                            nc.tensor.matmul(
                                psk[:],
                                wk_t[:, c, :],
                                cf_t[c][:, ibl * NB:(ibl + 1) * NB],
                                start=(c == 0), stop=(c == NCT - 1),
                            )
                        nc.scalar.activation(
                            out=k2[:, blk * NB:(blk + 1) * NB], in_=psk[:],
                            func=Ident, bias=bk_t[:], scale=1.0,
                        )
                    # vT projection: 16 j-tiles of this half
                    for jl in range(16):
                        j = h * 16 + jl
                        psv = pjv.tile([128, NB], F32, tag="pv", name=f"psv{j}")
                        for c in range(NCT):
                            nc.tensor.matmul(
                                psv[:],
                                cf_t[c][:, jl * 128:(jl + 1) * 128],
                                wvg_t[:, c, :],
                                start=(c == 0), stop=(c == NCT - 1),
                            )
                        nc.scalar.copy(v8[j // 2][:, j % 2, :], psv[:])
                    # Q projection: stream xf, accumulate 4 blocks in PSUM
                    psq = [
                        pjq.tile([128, NB], F32, tag=f"q{ibl}", name=f"psq{h}_{ibl}")
                        for ibl in range(4)
                    ]
                    for c in range(NCT):
                        xs = xfp.tile([128, 2048], BF16, tag="xs", name=f"xs{h}_{c}")
                        nc.sync.dma_start(out=xs, in_=xf[c * 128:(c + 1) * 128, csl])
                        for ibl in range(4):
                            nc.tensor.matmul(
                                psq[ibl][:],
                                wq_t[:, c, :],
                                xs[:, ibl * NB:(ibl + 1) * NB],
                                start=(c == 0), stop=(c == NCT - 1),
                            )
                    for ibl in range(4):
                        blk = h * 4 + ibl
                        nc.scalar.activation(
                            out=q2[:, blk * NB:(blk + 1) * NB], in_=psq[ibl][:],
                            func=Ident, bias=bq_t[:], scale=1.0,
                        )

            # ================= attention =================
            with (
                tc.tile_pool(name="pe", bufs=2, space="PSUM") as pe_pool,
                tc.tile_pool(name="po", bufs=2, space="PSUM") as po_pool,
                tc.tile_pool(name="pd", bufs=2, space="PSUM") as pd_pool,
                tc.tile_pool(name="pp", bufs=2) as pp,
                tc.tile_pool(name="att_io", bufs=4) as aio,
            ):
                p_cur = [None] * NPAIR  # p-tiles for the block being PV'd
                acc_cur = [None, None]

                def emit_energy_pair(b, t):
                    """energy + exp + denominator partial for pair (2t, 2t+1)."""
                    jA, jB = 2 * t, 2 * t + 1
                    isl = slice(b * NB, (b + 1) * NB)
                    pe = pe_pool.tile([128, 2 * NB], F32, tag="e", name=f"pe{b}_{t}")
                    nc.tensor.matmul(
                        pe[:, 0:NB],
                        k2[0:64, jA * 128:(jA + 1) * 128],
                        q2[0:64, isl],
                    )
                    nc.tensor.matmul(
                        pe[:, NB:2 * NB],
                        k2[64:128, jB * 128:(jB + 1) * 128],
                        q2[64:128, isl],
                    )
                    pt = pp.tile([128, 2 * NB], FP8, tag=f"p{t}", name=f"p{b}_{t}")
                    nc.scalar.activation(out=pt[:], in_=pe[:], func=Exp, bias=ebias_t[:], scale=1.0)
                    p_cur[t] = pt
                    # denominator: even pairs on DVE, odd pairs on PE (ones-matmuls)
                    if t == 0:
                        acc = aio.tile([128, NB], F32R, tag="acc", name=f"acc{b}")
                        acc_cur[0] = acc
                        nc.vector.tensor_add(acc[:], pt[:, 0:NB], pt[:, NB:2 * NB])
                    elif t % 2 == 0:
                        acc = acc_cur[0]
                        nc.vector.tensor_add(acc[:], acc[:], pt[:, 0:NB])
                        nc.vector.tensor_add(acc[:], acc[:], pt[:, NB:2 * NB])
                    else:
                        if t == 1:
                            acc_cur[1] = pd_pool.tile([1, NB], F32, tag="d", name=f"pd{b}")
                        pdn = acc_cur[1]
                        nc.tensor.matmul(pdn[:], ones_8[:], pt[:, 0:NB],
                                         start=(t == 1), stop=False)
                        nc.tensor.matmul(pdn[:], ones_8[:], pt[:, NB:2 * NB],
                                         start=False, stop=False)

                def emit_denominator(b, acc, pdn):
                    """Fold acc over partitions (ones-matmul), scatter, reciprocal."""
                    nc.tensor.matmul(pdn[:], ones_f[:], acc[:], start=False, stop=True)
                    den_sb = aio.tile([1, NB], F32, tag="den", name=f"den{b}")
                    nc.vector.tensor_copy(den_sb[:], pdn[:])
                    # partition-scatter via DRAM bounce: den[f*128+p] -> rcp[p, f]
                    nc.sync.dma_start(
                        out=den_d[b:b + 1].rearrange("o f p -> o (f p)"),
                        in_=den_sb[:],
                    )
                    rcp = aio.tile([128, 4], F32, tag="rcp", name=f"rcp{b}")
                    nc.sync.dma_start(
                        out=rcp[:], in_=den_d[b].rearrange("f p -> p f"),
                    )
                    rc4 = aio.tile([128, 4], F32, tag="rc4", name=f"rc4{b}")
                    nc.vector.reciprocal(rc4[:], rcp[:])
                    return rc4

                # prologue: block 0 energy
                for t in range(NPAIR):
                    emit_energy_pair(0, t)
                p_live = list(p_cur)
                acc_live, pd_live = acc_cur[0], acc_cur[1]

                for b in range(NBLK):
                    rc4 = None
                    # prefetch residual tiles for this block
                    xt_tiles = []
                    for it in range(4):
                        row = (b * 4 + it) * 128
                        xt = aio.tile([128, C], F32, tag="xt", name=f"xt{b}_{it}")
                        nc.sync.dma_start(out=xt, in_=xtr[row:row + 128, :])
                        xt_tiles.append(xt)

                    nxt = iter(range(NPAIR)) if b + 1 < NBLK else iter(())
                    pos = 0
                    for it in range(4):
                        po = po_pool.tile([128, NB], F32, tag="o", name=f"po{b}_{it}")
                        for k in range(NPAIR):
                            lhs = p_live[k].rearrange("p (h i) -> p h i", h=2)[
                                :, :, it * 128:(it + 1) * 128]
                            nc.tensor.matmul(
                                po[:], lhs, v8[k][:],
                                start=(k == 0), stop=(k == NPAIR - 1),
                                perf_mode=DR,
                            )
                            pos += 1
                            if pos % 4 == 0:
                                t_nxt = next(nxt, None)
                                if t_nxt is not None:
                                    emit_energy_pair(b + 1, t_nxt)
                        if rc4 is None:
                            rc4 = emit_denominator(b, acc_live, pd_live)
                        ot = aio.tile([128, C], F32, tag="ot", name=f"ot{b}_{it}")
                        nc.vector.scalar_tensor_tensor(
                            out=ot[:], in0=po[:], scalar=rc4[:, it:it + 1], in1=xt_tiles[it][:],
                            op0=mybir.AluOpType.mult, op1=mybir.AluOpType.add,
                        )
                        row = (b * 4 + it) * 128
                        nc.sync.dma_start(out=out_t[row:row + 128, :], in_=ot[:])
                    p_live = list(p_cur)
                    acc_live, pd_live = acc_cur[0], acc_cur[1]

    nc.finalize()
    return nc


_CACHE = {}


def _get_nc():
    if "nc" not in _CACHE:
        _CACHE["nc"] = build()
    return _CACHE["nc"]


def run(x, cond, wq, bq, wk, bk, wv, bv, gamma, trace=False, tmpdir=None):
    x = np.asarray(x, np.float32)
    cond = np.asarray(cond, np.float32)
    wq = np.asarray(wq, np.float32)
    bq = np.asarray(bq, np.float32)
    wk = np.asarray(wk, np.float32)
    bk = np.asarray(bk, np.float32)
    wv = np.asarray(wv, np.float32)
    bv = np.asarray(bv, np.float32)
    g = float(np.asarray(gamma, np.float32).reshape(-1)[0])

    import ml_dtypes
    BF = ml_dtypes.bfloat16

    wqtd = np.ascontiguousarray(np.concatenate([wq.T, wq.T], axis=1)).astype(BF)
    wktd = np.ascontiguousarray(np.concatenate([wk.T, wk.T], axis=1)).astype(BF)
    wvtg = np.ascontiguousarray((g * wv).T).astype(BF)                  # [C, C]
    bqd = np.ascontiguousarray(np.tile(bq, 2)[:, None])                 # [128, 1]
    bkd = np.ascontiguousarray(np.tile(bk, 2)[:, None])
    gbv = (g * bv)[None, :]                                             # [1, C]

    in_maps = []
    for b in range(B):
        xf32 = np.ascontiguousarray(x[b].reshape(C, N))
        xfb = xf32.astype(BF)
        cfb = np.ascontiguousarray(cond[b].reshape(C, N)).astype(BF)
        xtrb = np.ascontiguousarray(xf32.T + gbv)
        in_maps.append({
            "xf": xfb, "cf": cfb, "xtr": xtrb,
            "wqtd": wqtd, "wktd": wktd, "wvtg": wvtg,
            "bqd": bqd, "bkd": bkd,
        })

    nc = _get_nc()
    res = run_bass_kernel_spmd(
        nc, in_maps, list(range(B)), trace=trace, tmpdir=tmpdir,
    )
    out = np.empty((B, C, H, W), np.float32)
    for b in range(B):
        out[b] = res.results[b]["out_t"].T.reshape(C, H, W)
    return out, res


def kernel(**inputs):
    out, _ = run(**inputs)
    return out


# revision 24
# speedup vs baseline: 1.1671x; 1.0725x over previous
"""Trainium2 Bass kernel for nn_Atten_Cross: cross-attention block.

Per-core (data-parallel over batch, B=8 == 8 cores):
  xf  = x[b]    [C=512, N=4096]
  cf  = cond[b] [C, N]
  Q   = wq @ xf          [64, N]   (o on partitions, duplicated to 128)
  K   = wk @ cf          [64, N]   (duplicated to 128)
  vT  = (gamma*wv @ cf).T [N, C]   (j on partitions, bf16)
  eT  = K.T-tile @ Q     [N(j), N(i)]  computed per (j-tile, i-block), f32r
  P   = exp(eT)          (no max subtraction; |e| <~ 10 so exp is safe)
  outT[i, c] = sum_j P[j,i] vT[j,c] / sum_j P[j,i]
  out_t = outT + xT (+ gamma*bv folded into xT host-side)
Host transposes out_t back to [C, H, W].
"""

import numpy as np

import concourse.bass as bass
import concourse.mybir as mybir
from concourse import bacc
from concourse.tile import TileContext
from concourse.bass_utils import run_bass_kernel_spmd

B, C, H, W = 8, 512, 64, 64
N = H * W          # 4096
CQK = C // 8       # 64
NB = 512           # i-block (columns per energy/PV pass)
NBLK = N // NB     # 8 i-blocks
NJT = N // 128     # 32 j-tiles
NPAIR = NJT // 2   # 16 row-packed pairs
NCT = C // 128     # 4 contraction tiles

F32 = mybir.dt.float32
F32R = mybir.dt.float32r
BF16 = mybir.dt.bfloat16
Ident = mybir.ActivationFunctionType.Identity
Exp = mybir.ActivationFunctionType.Exp
FP8 = mybir.dt.float8e4
DR = mybir.MatmulPerfMode.DoubleRow
EXP_BIAS = -7.0


def build():
    nc = bacc.Bacc("TRN2", target_bir_lowering=False)

    xf = nc.dram_tensor("xf", [C, N], BF16, kind="ExternalInput")
    cf = nc.dram_tensor("cf", [C, N], BF16, kind="ExternalInput")
    xtr = nc.dram_tensor("xtr", [N, C], F32, kind="ExternalInput")
    wqtd = nc.dram_tensor("wqtd", [C, 128], BF16, kind="ExternalInput")
    wktd = nc.dram_tensor("wktd", [C, 128], BF16, kind="ExternalInput")
    wvtg = nc.dram_tensor("wvtg", [C, C], BF16, kind="ExternalInput")
    bqd = nc.dram_tensor("bqd", [128, 1], F32, kind="ExternalInput")
    bkd = nc.dram_tensor("bkd", [128, 1], F32, kind="ExternalInput")
    out_t = nc.dram_tensor("out_t", [N, C], F32, kind="ExternalOutput")
    den_d = nc.dram_tensor("den_scratch", [NBLK, 4, 128], F32)

    with TileContext(nc) as tc:
        with (
            tc.tile_pool(name="wp", bufs=1) as wp,
            tc.tile_pool(name="qk", bufs=1) as qk,
            tc.tile_pool(name="vp", bufs=1) as vp,
            tc.tile_pool(name="io", bufs=1) as io,
        ):
            # ---- weights / constants ----
            wq_t = wp.tile([128, NCT, 128], BF16)
            wk_t = wp.tile([128, NCT, 128], BF16)
            wvg_t = wp.tile([128, NCT, C], BF16)
            for c in range(NCT):
                nc.sync.dma_start(out=wq_t[:, c, :], in_=wqtd[c * 128:(c + 1) * 128, :])
                nc.sync.dma_start(out=wk_t[:, c, :], in_=wktd[c * 128:(c + 1) * 128, :])
                nc.sync.dma_start(out=wvg_t[:, c, :], in_=wvtg[c * 128:(c + 1) * 128, :])
            bq_t = wp.tile([128, 1], F32)
            bk_t = wp.tile([128, 1], F32)
            nc.sync.dma_start(out=bq_t, in_=bqd[:])
            nc.sync.dma_start(out=bk_t, in_=bkd[:])
            ones_f = wp.tile([128, 1], F32)
            nc.vector.memset(ones_f, 1.0)
            ebias_t = wp.tile([128, 1], F32)
            nc.vector.memset(ebias_t, EXP_BIAS)

            q2 = qk.tile([128, N], BF16)   # [o-dup, i]
            k2 = qk.tile([128, N], BF16)   # [o-dup, j]
            v8 = [vp.tile([128, 2, NB], FP8, tag=f"v{t}", name=f"v{t}") for t in range(NPAIR)]

            # ================= projections =================
            with (
                tc.tile_pool(name="cfp", bufs=2) as cfp,
                tc.tile_pool(name="xfp", bufs=2) as xfp,
                tc.tile_pool(name="pjq", bufs=1, space="PSUM") as pjq,
                tc.tile_pool(name="pjv", bufs=2, space="PSUM") as pjv,
            ):
                for h in range(2):
                    csl = slice(h * 2048, (h + 1) * 2048)
                    cf_t = []
                    for c in range(NCT):
                        t = cfp.tile([128, 2048], BF16, tag=f"c{c}", name=f"cf{c}_{h}")
                        nc.sync.dma_start(out=t, in_=cf[c * 128:(c + 1) * 128, csl])
                        cf_t.append(t)
                    # K projection: 4 column-blocks of this half
                    for ibl in range(4):
                        blk = h * 4 + ibl
                        psk = pjq.tile([128, NB], F32, tag=f"q{ibl}", name=f"psk{blk}")
                        for c in range(NCT):
                            nc.tensor.matmul(
                                psk[:],
                                wk_t[:, c, :],
                                cf_t[c][:, ibl * NB:(ibl + 1) * NB],
                                start=(c == 0), stop=(c == NCT - 1),
                            )
                        nc.scalar.activation(
                            out=k2[:, blk * NB:(blk + 1) * NB], in_=psk[:],
                            func=Ident, bias=bk_t[:], scale=1.0,
                        )
                    # vT projection: 16 j-tiles of this half
                    for jl in range(16):
                        j = h * 16 + jl
                        psv = pjv.tile([128, NB], F32, tag="pv", name=f"psv{j}")
                        for c in range(NCT):
                            nc.tensor.matmul(
                                psv[:],
                                cf_t[c][:, jl * 128:(jl + 1) * 128],
                                wvg_t[:, c, :],
                                start=(c == 0), stop=(c == NCT - 1),
                            )
                        nc.vector.tensor_copy(v8[j // 2][:, j % 2, :], psv[:])
                    # Q projection: stream xf, accumulate 4 blocks in PSUM
                    psq = [
                        pjq.tile([128, NB], F32, tag=f"q{ibl}", name=f"psq{h}_{ibl}")
                        for ibl in range(4)
                    ]
                    for c in range(NCT):
                        xs = xfp.tile([128, 2048], BF16, tag="xs", name=f"xs{h}_{c}")
                        nc.sync.dma_start(out=xs, in_=xf[c * 128:(c + 1) * 128, csl])
                        for ibl in range(4):
                            nc.tensor.matmul(
                                psq[ibl][:],
                                wq_t[:, c, :],
                                xs[:, ibl * NB:(ibl + 1) * NB],
                                start=(c == 0), stop=(c == NCT - 1),
                            )
                    for ibl in range(4):
                        blk = h * 4 + ibl
                        nc.scalar.activation(
                            out=q2[:, blk * NB:(blk + 1) * NB], in_=psq[ibl][:],
                            func=Ident, bias=bq_t[:], scale=1.0,
                        )

            # ================= attention =================
            with (
                tc.tile_pool(name="pe", bufs=2, space="PSUM") as pe_pool,
                tc.tile_pool(name="po", bufs=2, space="PSUM") as po_pool,
                tc.tile_pool(name="pd", bufs=2, space="PSUM") as pd_pool,
                tc.tile_pool(name="pp", bufs=2) as pp,
                tc.tile_pool(name="att_io", bufs=4) as aio,
            ):
                p_cur = [None] * NPAIR  # p-tiles for the block being PV'd
                acc_cur = [None]

                def emit_energy_pair(b, t):
                    """energy + exp + denominator partial for pair (2t, 2t+1)."""
                    jA, jB = 2 * t, 2 * t + 1
                    isl = slice(b * NB, (b + 1) * NB)
                    pe = pe_pool.tile([128, 2 * NB], F32, tag="e", name=f"pe{b}_{t}")
                    nc.tensor.matmul(
                        pe[:, 0:NB],
                        k2[0:64, jA * 128:(jA + 1) * 128],
                        q2[0:64, isl],
                    )
                    nc.tensor.matmul(
                        pe[:, NB:2 * NB],
                        k2[64:128, jB * 128:(jB + 1) * 128],
                        q2[64:128, isl],
                    )
                    pt = pp.tile([128, 2 * NB], FP8, tag=f"p{t}", name=f"p{b}_{t}")
                    nc.scalar.activation(out=pt[:], in_=pe[:], func=Exp, bias=ebias_t[:], scale=1.0)
                    p_cur[t] = pt
                    # DVE denominator accumulation: acc[p, i] += P[.., i] slices
                    if t == 0:
                        acc = aio.tile([128, NB], F32, tag="acc", name=f"acc{b}")
                        acc_cur[0] = acc
                        nc.vector.tensor_add(acc[:], pt[:, 0:NB], pt[:, NB:2 * NB])
                    else:
                        acc = acc_cur[0]
                        nc.vector.tensor_add(acc[:], acc[:], pt[:, 0:NB])
                        nc.vector.tensor_add(acc[:], acc[:], pt[:, NB:2 * NB])

                def emit_denominator(b, acc):
                    """Fold acc over partitions (ones-matmul), scatter, reciprocal."""
                    pdn = pd_pool.tile([1, NB], F32, tag="d", name=f"pd{b}")
                    nc.tensor.matmul(pdn[:], ones_f[:], acc[:])
                    den_sb = aio.tile([1, NB], F32, tag="den", name=f"den{b}")
                    nc.vector.tensor_copy(den_sb[:], pdn[:])
                    # partition-scatter via DRAM bounce: den[f*128+p] -> rcp[p, f]
                    nc.sync.dma_start(
                        out=den_d[b:b + 1].rearrange("o f p -> o (f p)"),
                        in_=den_sb[:],
                    )
                    rcp = aio.tile([128, 4], F32, tag="rcp", name=f"rcp{b}")
                    nc.sync.dma_start(
                        out=rcp[:], in_=den_d[b].rearrange("f p -> p f"),
                    )
                    rc4 = aio.tile([128, 4], F32, tag="rc4", name=f"rc4{b}")
                    nc.vector.reciprocal(rc4[:], rcp[:])
                    return rc4

                # prologue: block 0 energy
                for t in range(NPAIR):
                    emit_energy_pair(0, t)
                p_live = list(p_cur)
                acc_live = acc_cur[0]

                for b in range(NBLK):
                    rc4 = emit_denominator(b, acc_live)
                    # prefetch residual tiles for this block
                    xt_tiles = []
                    for it in range(4):
                        row = (b * 4 + it) * 128
                        xt = aio.tile([128, C], F32, tag="xt", name=f"xt{b}_{it}")
                        nc.sync.dma_start(out=xt, in_=xtr[row:row + 128, :])
                        xt_tiles.append(xt)

                    nxt = iter(range(NPAIR)) if b + 1 < NBLK else iter(())
                    pos = 0
                    for it in range(4):
                        po = po_pool.tile([128, NB], F32, tag="o", name=f"po{b}_{it}")
                        for k in range(NPAIR):
                            lhs = p_live[k].rearrange("p (h i) -> p h i", h=2)[
                                :, :, it * 128:(it + 1) * 128]
                            nc.tensor.matmul(
                                po[:], lhs, v8[k][:],
                                start=(k == 0), stop=(k == NPAIR - 1),
                                perf_mode=DR,
                            )
                            pos += 1
                            if pos % 4 == 0:
                                t_nxt = next(nxt, None)
                                if t_nxt is not None:
                                    emit_energy_pair(b + 1, t_nxt)
                        ot = aio.tile([128, C], F32, tag="ot", name=f"ot{b}_{it}")
                        nc.vector.scalar_tensor_tensor(
                            out=ot[:], in0=po[:], scalar=rc4[:, it:it + 1], in1=xt_tiles[it][:],
                            op0=mybir.AluOpType.mult, op1=mybir.AluOpType.add,
                        )
                        row = (b * 4 + it) * 128
                        nc.sync.dma_start(out=out_t[row:row + 128, :], in_=ot[:])
                    p_live = list(p_cur)
                    acc_live = acc_cur[0]

    nc.finalize()
    return nc


_CACHE = {}


def _get_nc():
    if "nc" not in _CACHE:
        _CACHE["nc"] = build()
    return _CACHE["nc"]


def run(x, cond, wq, bq, wk, bk, wv, bv, gamma, trace=False, tmpdir=None):
    x = np.asarray(x, np.float32)
    cond = np.asarray(cond, np.float32)
    wq = np.asarray(wq, np.float32)
    bq = np.asarray(bq, np.float32)
    wk = np.asarray(wk, np.float32)
    bk = np.asarray(bk, np.float32)
    wv = np.asarray(wv, np.float32)
    bv = np.asarray(bv, np.float32)
    g = float(np.asarray(gamma, np.float32).reshape(-1)[0])

    import ml_dtypes
    BF = ml_dtypes.bfloat16

    wqtd = np.ascontiguousarray(np.concatenate([wq.T, wq.T], axis=1)).astype(BF)
    wktd = np.ascontiguousarray(np.concatenate([wk.T, wk.T], axis=1)).astype(BF)
    wvtg = np.ascontiguousarray((g * wv).T).astype(BF)                  # [C, C]
    bqd = np.ascontiguousarray(np.tile(bq, 2)[:, None])                 # [128, 1]
    bkd = np.ascontiguousarray(np.tile(bk, 2)[:, None])
    gbv = (g * bv)[None, :]                                             # [1, C]

    in_maps = []
    for b in range(B):
        xf32 = np.ascontiguousarray(x[b].reshape(C, N))
        xfb = xf32.astype(BF)
        cfb = np.ascontiguousarray(cond[b].reshape(C, N)).astype(BF)
        xtrb = np.ascontiguousarray(xf32.T + gbv)
        in_maps.append({
            "xf": xfb, "cf": cfb, "xtr": xtrb,
            "wqtd": wqtd, "wktd": wktd, "wvtg": wvtg,
            "bqd": bqd, "bkd": bkd,
        })

    nc = _get_nc()
    res = run_bass_kernel_spmd(
        nc, in_maps, list(range(B)), trace=trace, tmpdir=tmpdir,
    )
    out = np.empty((B, C, H, W), np.float32)
    for b in range(B):
        out[b] = res.results[b]["out_t"].T.reshape(C, H, W)
    return out, res


def kernel(**inputs):
    out, _ = run(**inputs)
    return out
